# revision 2
# baseline (speedup 1.0000x reference)
"""Distributed Trainium2 kernel for LoRA-fused GQA attention.

Sharding: 8 cores = 2 (batch) x 4 (head-pairs). Core 4*b+j computes batch b,
heads {2j, 2j+1} end-to-end (q-proj, RoPE, causal attention vs the shared
KV head, partial out-proj); the host sums the 4 partial out-projections per
batch. LoRA adapters are folded into effective projection weights on the host
(exact algebraic rewrite; the einsum's repeated head index is a diagonal).

The shared K head is sequence-parallel: each core projects+ropes only its
own 512-key chunk and an on-device AllGather (per 4-core batch group)
assembles the full K while the q-projection wave hides the collective
latency. V stays replicated (a second gather cannot be hidden). The softmax
row-sum runs off the PE: p tiles accumulate on the DVE and are
partition-reduced + broadcast in one GPSIMD op.

Self-contained: hardcodes all shapes from the problem spec.
"""

import os

import numpy as np
import ml_dtypes

B, T, D, N, K, H, L = 2, 2048, 2048, 8, 1, 256, 16
LORA_SCALE = 16.0 / 16
BIG_NEG = -2.3819763e38
N_CORES = 8
P = 128
NH = N * H  # 2048
TJ = 512  # t-chunk width (free dim of logits/out tiles)
NTJ = T // TJ  # 4
NST = T // P  # 16 s-tiles
DCH = D // P  # 16 contraction chunks
HEADS_PER_CORE = 2
NHC = HEADS_PER_CORE * H // P  # 4 q/enc row-tiles of 128 per core

LAST_EXEC_TIME_NS = None

_BF16 = ml_dtypes.bfloat16


def _fold_weights(q_w, q_lora_a, q_lora_b, kv_w, kv_lora_a, kv_lora_b,
                  out_w, out_lora_a, out_lora_b):
    """Fold LoRA into effective dense weights (all float32 math)."""
    # q: [N,D,H] + a[N,D,L] @ diag_b[N,L,H] -> Wq_eff [D, N*H]
    bd = np.stack([q_lora_b[n, :, n, :] for n in range(N)])  # [N,L,H]
    wq = q_w + np.einsum("ndl,nlh->ndh", q_lora_a, bd) * LORA_SCALE
    wq_eff = wq.transpose(1, 0, 2).reshape(D, NH).astype(np.float32)
    # kv: [2,K,D,H], K=1
    kvb = kv_lora_b[:, 0, :, 0, :]  # [2,L,H]
    wkv = kv_w[:, 0] + np.einsum("idl,ilh->idh", kv_lora_a[:, 0], kvb) * LORA_SCALE
    wk_eff = wkv[0].astype(np.float32)  # [D,H]
    wv_eff = wkv[1].astype(np.float32)  # [D,H]
    # out: [N,H,D] + diag_a[N,H,L] @ b[N,L,D] -> Wo_eff [N*H, D]
    ad = np.stack([out_lora_a[n, :, n, :] for n in range(N)])  # [N,H,L]
    wo = out_w + np.einsum("nhl,nld->nhd", ad, out_lora_b) * LORA_SCALE
    wo_eff = wo.reshape(NH, D).astype(np.float32)
    return wq_eff, wk_eff, wv_eff, wo_eff


def _rope_tables(positions_b):
    """cos/sin [P, T] f32 for one batch row of positions."""
    freq_exp = (2.0 / H) * np.arange(H // 2, dtype=np.float32)
    timescale = (10000.0 ** freq_exp).astype(np.float32)  # [128]
    rad = positions_b.astype(np.float32)[None, :] / timescale[:, None]  # [128,T]
    return np.cos(rad).astype(np.float32), np.sin(rad).astype(np.float32)


def _classify_mask(attn_mask):
    """Tile-classify the (shared-program) mask.

    Returns (s_tiles, lo_cols, bias_list):
      s_tiles[tj]: ordered 128-wide s-tile indices to process for t-chunk tj
      lo_cols[(tj, st)]: leading all-false column count (memset'd, skipped
        by exp) in the [128 s, TJ t] logits tile
      bias_list: (tj, st, mcol) 128-col sub-blocks needing an additive bias
    Inclusion is the union over both batches so the SPMD program is
    identical on every core; per-core bias data covers the rest.
    """
    m = np.asarray(attn_mask)[:, 0]  # [B, T(query), S(key)]
    s_tiles = []
    lo_cols = {}
    bias_list = []
    for tj in range(NTJ):
        tiles = []
        for st in range(NST):
            sk = slice(st * P, (st + 1) * P)
            subs = [m[:, tj * TJ + mc * P:tj * TJ + (mc + 1) * P, sk]
                    for mc in range(TJ // P)]
            if not any(s.any() for s in subs):
                continue
            tiles.append(st)
            lo = 0
            while lo < len(subs) and not subs[lo].any():
                lo += 1
            lo_cols[(tj, st)] = lo * P
            for mc in range(lo, len(subs)):
                if not subs[mc].all():
                    bias_list.append((tj, st, mc))
        s_tiles.append(tiles)
    return s_tiles, lo_cols, bias_list


def _build_program(s_tiles, lo_cols, bias_list):
    """Build + compile the single SPMD Bass program."""
    from contextlib import ExitStack

    import concourse.tile as tile
    from concourse import bacc, mybir
    from concourse.bass_isa import ReduceOp

    bf16 = mybir.dt.bfloat16
    f32 = mybir.dt.float32
    AF = mybir.ActivationFunctionType

    nc = bacc.Bacc("TRN2", target_bir_lowering=False, debug=False,
                   num_devices=N_CORES)

    xT_e = nc.dram_tensor("xT", [DCH, P, T], bf16, kind="ExternalInput")
    xown_e = nc.dram_tensor("xown", [DCH, P, TJ], bf16, kind="ExternalInput")
    wq_e = nc.dram_tensor("wq", [NHC, P, DCH * P], bf16, kind="ExternalInput")
    wk_e = nc.dram_tensor("wk", [2, P, DCH * P], bf16, kind="ExternalInput")
    wv_e = nc.dram_tensor("wv", [DCH, P, H], bf16, kind="ExternalInput")
    wo_e = nc.dram_tensor("wo", [DCH, P, NHC * P], bf16, kind="ExternalInput")
    csq_e = nc.dram_tensor("csq", [2, P, T], bf16, kind="ExternalInput")
    cskown_e = nc.dram_tensor("cskown", [2, P, TJ], bf16,
                              kind="ExternalInput")
    nbias = max(1, len(bias_list))
    bias_e = nc.dram_tensor("bias", [nbias, P, P], bf16, kind="ExternalInput")
    out_e = nc.dram_tensor("out", [DCH, P, T], bf16, kind="ExternalOutput")

    # K sequence-parallel exchange: each core in a 4-core batch group
    # computes (and ropes) k for its own 512-key chunk; one AllGather
    # shares them. V stays replicated: its gather would double the
    # collective size past what the q-projection phase can hide.
    cc_in = nc.dram_tensor("cc_in", [2, P, TJ], bf16)
    cc_out = nc.dram_tensor("cc_out", [4, 2, P, TJ], bf16)
    cc_groups = [[0, 1, 2, 3], [4, 5, 6, 7]]

    bias_idx = {k: i for i, k in enumerate(bias_list)}

    with tile.TileContext(nc) as tc:
        with ExitStack() as ctx:
            sb = ctx.enter_context(tc.tile_pool(name="sb", bufs=1))
            ps = ctx.enter_context(tc.tile_pool(name="ps", bufs=1, space="PSUM"))

            # ---- loads, in consumption order: wq/wk -> xT -> cs -> wv/wo/bias
            wq_sb = []
            for n in range(NHC):
                t_ = sb.tile([P, DCH * P], bf16, tag="wq", bufs=NHC,
                             name=f"wqs{n}")
                nc.sync.dma_start(t_[:, :], wq_e.ap()[n])
                wq_sb.append(t_)
            wk_sb = []
            for j in range(2):
                t_ = sb.tile([P, DCH * P], bf16, tag="wk", bufs=2,
                             name=f"wks{j}")
                nc.sync.dma_start(t_[:, :], wk_e.ap()[j])
                wk_sb.append(t_)
            xt = []
            for c in range(DCH):
                t_ = sb.tile([P, T], bf16, tag="xs", bufs=DCH, name=f"xt{c}")
                nc.sync.dma_start(t_[:, :], xT_e.ap()[c])
                xt.append(t_)
            # xq (the core's own 512 t-cols, for the k-chunk proj) streams
            # AFTER the full xT: wave 1 is q-only, so front-loading xq would
            # just dilute the x stream the wave is gated on.
            xq = []
            for c in range(DCH):
                t2 = sb.tile([P, TJ], bf16, tag="xq", bufs=DCH,
                             name=f"xq{c}")
                nc.sync.dma_start(t2[:, :], xown_e.ap()[c])
                xq.append(t2)
            csko = []
            for i in range(2):
                t_ = sb.tile([P, TJ], bf16, tag="csko", bufs=2,
                             name=f"csko{i}")
                nc.sync.dma_start(t_[:, :], cskown_e.ap()[i])
                csko.append(t_)
            wv_sb = []
            for c in range(DCH):
                t_ = sb.tile([P, H], bf16, tag="wv", bufs=DCH, name=f"wv{c}")
                nc.sync.dma_start(t_[:, :], wv_e.ap()[c])
                wv_sb.append(t_)
            csq = []
            for i in range(2):
                t_ = sb.tile([P, T], bf16, tag="cs", bufs=2, name=f"csq{i}")
                nc.sync.dma_start(t_[:, :], csq_e.ap()[i])
                csq.append(t_)
            wo_sb = []
            for dt in range(DCH):
                t_ = sb.tile([P, NHC * P], bf16, tag="wo", bufs=DCH,
                             name=f"wos{dt}")
                nc.sync.dma_start(t_[:, :], wo_e.ap()[dt])
                wo_sb.append(t_)
            bias_sb = {}
            for key in bias_list:
                tj, st, mc = key
                t_ = sb.tile([P, P], bf16, tag="bias", bufs=nbias,
                             name=f"bias{tj}_{st}_{mc}")
                nc.sync.dma_start(t_[:, :], bias_e.ap()[bias_idx[key]])
                bias_sb[key] = t_
            ones_col = sb.tile([P, 1], bf16, tag="small", bufs=4)
            nc.vector.memset(ones_col[:, :], 1.0)

            # PE warm-up during the initial DMA wait: ~5us of throwaway
            # matmuls so the HAM clock-gate opens before real work lands.
            warm_src = sb.tile([P, TJ], bf16, tag="p", bufs=12, name="warm")
            nc.vector.memset(warm_src[:, :], 0.0)
            wps = ps.tile([1, TJ], f32, tag="lg", bufs=3, name="warmps")
            for i in range(20):
                nc.tensor.matmul(wps[:, :], ones_col[:, :], warm_src[:, :],
                                 start=(i == 0), stop=False)

            # shared rope temporaries: one 512-col chunk at a time, so
            # they cost 4KB of SBUF instead of 16KB (freed for deeper
            # p/outc/normalizer rings)
            rt = [sb.tile([P, TJ], bf16, tag="rt", bufs=4, name=f"rt{i}")
                  for i in range(4)]

            def rope_pair(dst0, dst1, src0, src1, cs, c0=0, c1=T):
                """dst0 = s0*cos - s1*sin ; dst1 = s1*cos + s0*sin.

                In-place safe (dst may alias src): all four products are
                computed into temps before the writes. [c0:c1] columns,
                at most TJ wide per call.
                """
                assert c1 - c0 <= TJ
                cos_t, sin_t = cs
                sl = slice(c0, c1)
                tl = slice(0, c1 - c0)
                a, bm, c2, d2 = rt
                nc.vector.tensor_mul(a[:, tl], src0[:, sl], cos_t[:, sl])
                nc.vector.tensor_mul(bm[:, tl], src1[:, sl], sin_t[:, sl])
                nc.vector.tensor_mul(c2[:, tl], src1[:, sl], cos_t[:, sl])
                nc.vector.tensor_mul(d2[:, tl], src0[:, sl], sin_t[:, sl])
                nc.vector.tensor_sub(dst0[:, sl], a[:, tl], bm[:, tl])
                nc.vector.tensor_add(dst1[:, sl], c2[:, tl], d2[:, tl])

            # ---- stage A wave 1: first 7 q groups ----
            # Every accumulation group consumes each x chunk the moment its
            # DMA lands (in-order PE: group-major would stall everything
            # behind group 0's last chunk).
            atags = [("ps512", 2), ("pse", 3), ("pse", 3), ("lg", 3),
                     ("ps512", 2), ("pse", 3), ("lg", 3)]
            qraw = [sb.tile([P, T], bf16, tag="qk", bufs=4, name=f"qraw{n}")
                    for n in range(NHC)]
            qgroups = [(n, t4) for t4 in range(T // 512) for n in range(NHC)]
            wave1 = qgroups[:len(atags)]
            pq1 = {}
            for gidx, (n, t4) in enumerate(wave1):
                tg, nb = atags[gidx]
                pq1[(n, t4)] = ps.tile([P, 512], f32, tag=tg, bufs=nb,
                                       name="pq")
            for c in range(DCH):
                for (n, t4) in wave1:
                    nc.tensor.matmul(
                        pq1[(n, t4)][:, :],
                        wq_sb[n][:, c * P:(c + 1) * P],
                        xt[c][:, t4 * 512:(t4 + 1) * 512],
                        start=(c == 0), stop=(c == DCH - 1),
                    )
                # filler: keep the PE fed while the next x chunk lands
                nc.tensor.matmul(wps[:, :], ones_col[:, :],
                                 warm_src[:, :], start=False,
                                 stop=(c == DCH - 1))
            # early copies free the PSUM slots the k-chunk groups and the
            # first v-proj groups take (ACT runs these while the PE chews
            # through the k-chunk phase)
            for (n, t4) in [(0, 0), (1, 0), (2, 0), (3, 0), (0, 1)]:
                nc.any.tensor_copy(qraw[n][:, t4 * 512:(t4 + 1) * 512],
                                   pq1[(n, t4)][:, :])

            # ---- k-chunk proj (own 512 keys), rope, pack, AllGather ----
            # xq streams in right after xT; the two groups consume it
            # chunk-major as it lands, with one q group interleaved so the
            # PE isn't gated on the thinner xq DMA stream.
            kck = [ps.tile([P, TJ], f32, tag=tg, bufs=nb, name=f"pkc{j}")
                   for j, (tg, nb) in enumerate([("ps512", 2), ("pse", 3)])]
            pq02 = ps.tile([P, 512], f32, tag="lg", bufs=3, name="pq02")
            for c in range(DCH):
                for jh in range(2):
                    nc.tensor.matmul(
                        kck[jh][:, :],
                        wk_sb[jh][:, c * P:(c + 1) * P],
                        xq[c][:, :],
                        start=(c == 0), stop=(c == DCH - 1),
                    )
                nc.tensor.matmul(
                    pq02[:, :],
                    wq_sb[0][:, c * P:(c + 1) * P],
                    xt[c][:, 2 * 512:3 * 512],
                    start=(c == 0), stop=(c == DCH - 1),
                )
            kc = [sb.tile([P, TJ], bf16, tag="kc", bufs=2, name=f"kc{j}")
                  for j in range(2)]
            for jh in range(2):
                nc.any.tensor_copy(kc[jh][:, :], kck[jh][:, :])
            rope_pair(kc[0], kc[1], kc[0], kc[1], csko, c1=TJ)
            nc.gpsimd.dma_start(cc_in.ap()[0], kc[0][:, :])
            nc.gpsimd.dma_start(cc_in.ap()[1], kc[1][:, :])
            nc.gpsimd.collective_compute(
                "AllGather", mybir.AluOpType.bypass, replica_groups=cc_groups,
                ins=[cc_in.ap().opt()], outs=[cc_out.ap().opt()])
            # remaining wave-1 copies, in slot-freeing order for wave 2a
            for (n, t4) in [(1, 1), (2, 1)]:
                nc.any.tensor_copy(qraw[n][:, t4 * 512:(t4 + 1) * 512],
                                   pq1[(n, t4)][:, :])
            nc.any.tensor_copy(qraw[0][:, 2 * 512:3 * 512], pq02[:, :])

            # ---- wave 2a: v projection (replicated; all 16 s-tiles) ----
            # each group's tag matches the order in which earlier slots are
            # freed by the copy stream above, so the PE never waits long
            atags2 = [("pse", 3), ("lg", 3), ("ps512", 2), ("pse", 3),
                      ("lg", 3), ("ps512", 2), ("pse", 3)]
            gi = 0
            v_sb = []
            vps = []
            # first two groups run chunk-major together: the group is gated
            # on the just-arriving wv stream, and one group alone consumes
            # it slower than it lands
            for st in range(2):
                tg, nb = atags2[gi % len(atags2)]
                gi += 1
                vps.append(ps.tile([P, H], f32, tag=tg, bufs=nb, name="pv"))
            for c in range(DCH):
                for st in range(2):
                    nc.tensor.matmul(
                        vps[st][:, :],
                        xt[c][:, st * P:(st + 1) * P],
                        wv_sb[c][:, :],
                        start=(c == 0), stop=(c == DCH - 1),
                    )
            for st in range(2):
                vt = sb.tile([P, H], bf16, tag="xq", bufs=DCH, name=f"v{st}")
                nc.scalar.copy(vt[:, :], vps[st][:, :])
                v_sb.append(vt)
            for st in range(2, NST):
                tg, nb = atags2[gi % len(atags2)]
                gi += 1
                pv = ps.tile([P, H], f32, tag=tg, bufs=nb, name="pv")
                for c in range(DCH):
                    nc.tensor.matmul(
                        pv[:, :],
                        xt[c][:, st * P:(st + 1) * P],
                        wv_sb[c][:, :],
                        start=(c == 0), stop=(c == DCH - 1),
                    )
                vt = sb.tile([P, H], bf16, tag="xq", bufs=DCH, name=f"v{st}")
                nc.scalar.copy(vt[:, :], pv[:, :])
                v_sb.append(vt)

            # ---- wave 2b: remaining q groups (x resident) ----
            # n-major so each head-pair's qraw completes (and ropes, in
            # place) as early as possible — the first QK only needs pair 0.
            # Tag order keeps 'lg' (first QK) and 'pse' (e0/e1) out of the
            # last slots so attention isn't gated on the final copies.
            wave2b_tags = [("pse", 3), ("lg", 3), ("ps512", 2), ("pse", 3),
                           ("lg", 3), ("pse", 3), ("ps512", 2), ("ps512", 2)]
            q = qraw
            done = set(wave1) | {(0, 2)}
            w2i = 0
            for n in range(NHC):
                for t4 in range(T // 512):
                    if (n, t4) in done:
                        continue
                    tg, nb = wave2b_tags[w2i]
                    w2i += 1
                    pq = ps.tile([P, 512], f32, tag=tg, bufs=nb, name="pq")
                    for c in range(DCH):
                        nc.tensor.matmul(
                            pq[:, :],
                            wq_sb[n][:, c * P:(c + 1) * P],
                            xt[c][:, t4 * 512:(t4 + 1) * 512],
                            start=(c == 0), stop=(c == DCH - 1),
                        )
                    nc.any.tensor_copy(qraw[n][:, t4 * 512:(t4 + 1) * 512],
                                       pq[:, :])
                if n == 1:
                    # pair 0 ropes as soon as its qraw is complete
                    for rc in range(NTJ):
                        rope_pair(q[0], q[1], qraw[0], qraw[1], csq,
                                  c0=rc * TJ, c1=(rc + 1) * TJ)
            # pair 1: rope only the first t-chunk now; the rest is emitted
            # lazily inside the attention loop so tj=0's bias adds and p
            # accumulation aren't stuck behind 5us of rope on the in-order
            # DVE queue (hh=1 of chunk tj only needs columns tj*TJ onward).
            rope_pair(q[2], q[3], qraw[2], qraw[3], csq, c0=0, c1=TJ)

            # ---- unpack the gathered k (gpsimd queue: serialized after
            # the collective, no cross-engine sem churn) ----
            k0 = sb.tile([P, T], bf16, tag="wk", bufs=2, name="k0")
            k1 = sb.tile([P, T], bf16, tag="wk", bufs=2, name="k1")
            for j in range(4):
                nc.gpsimd.dma_start(k0[:, j * TJ:(j + 1) * TJ],
                                    cc_out.ap()[j][0])
                nc.gpsimd.dma_start(k1[:, j * TJ:(j + 1) * TJ],
                                    cc_out.ap()[j][1])

            # ---- stage C+D: attention per (head, t-chunk), fused out-proj --
            enc = []
            for n in range(NHC):
                # tag "xs": reuse the xT slots (dead after the v projection)
                en = sb.tile([P, T], bf16, tag="xs", bufs=DCH, name=f"enc{n}")
                enc.append(en)
            def outproj(tj):
                tsl = slice(tj * TJ, (tj + 1) * TJ)
                for dt in range(DCH):
                    po = ps.tile([P, TJ], f32, tag="ps512", bufs=2, name="po")
                    for c in range(NHC):
                        nc.tensor.matmul(
                            po[:, :],
                            wo_sb[dt][:, c * P:(c + 1) * P],
                            enc[c][:, tsl],
                            start=(c == 0), stop=(c == NHC - 1),
                        )
                    oc = sb.tile([P, TJ], bf16, tag="outc", bufs=16, name="oc")
                    # last two copies on DVE: the next chunk's first exp is
                    # queued behind these on the in-order ACT engine
                    if dt >= DCH - 2:
                        nc.vector.tensor_copy(oc[:, :], po[:, :])
                    else:
                        nc.any.tensor_copy(oc[:, :], po[:, :])
                    nc.sync.dma_start(out_e.ap()[dt][:, tsl], oc[:, :])

            for tj in range(NTJ):
                tsl = slice(tj * TJ, (tj + 1) * TJ)
                tiles = s_tiles[tj]
                for hh in range(HEADS_PER_CORE):
                    if not tiles:
                        for j in range(2):
                            nc.vector.memset(enc[2 * hh + j][:, tsl], 0.0)
                        continue
                    e0 = ps.tile([P, TJ], f32, tag="pse", bufs=3, name="e0")
                    e1 = ps.tile([P, TJ], f32, tag="pse", bufs=3, name="e1")
                    psum_t = sb.tile([P, TJ], f32, tag="lsum", bufs=2,
                                     name="psum")
                    last = len(tiles) - 1

                    def emit_pv(idx, st, lo, p_sb):
                        nc.tensor.matmul(e0[:, lo:], v_sb[st][:, 0:P],
                                         p_sb[:, lo:],
                                         start=(idx == 0), stop=(idx == last))
                        nc.tensor.matmul(e1[:, lo:], v_sb[st][:, P:H],
                                         p_sb[:, lo:],
                                         start=(idx == 0), stop=(idx == last))

                    # software pipeline: PV/l of tile st-1 are emitted after
                    # QK of tile st, so the in-order PE never waits on exp
                    pv_q = []
                    for idx, st in enumerate(tiles):
                        ssl = slice(st * P, (st + 1) * P)
                        # The first tile runs full-width so every e/l column
                        # has its accumulation group opened (start=True).
                        lo = 0 if idx == 0 else lo_cols[(tj, st)]
                        lg = ps.tile([P, TJ], f32, tag="lg", bufs=3, name="lg")
                        nc.tensor.matmul(lg[:, lo:], k0[:, ssl],
                                         q[2 * hh][:, tj * TJ + lo:
                                                   (tj + 1) * TJ],
                                         start=True, stop=False)
                        nc.tensor.matmul(lg[:, lo:], k1[:, ssl],
                                         q[2 * hh + 1][:, tj * TJ + lo:
                                                       (tj + 1) * TJ],
                                         start=False, stop=True)
                        for mc in range(lo // P, TJ // P):
                            key = (tj, st, mc)
                            if key in bias_sb:
                                msl = slice(mc * P, (mc + 1) * P)
                                nc.vector.tensor_add(lg[:, msl], lg[:, msl],
                                                     bias_sb[key][:, :])
                        p_sb = sb.tile([P, TJ], bf16, tag="p", bufs=12,
                                       name="psb")
                        nc.scalar.activation(p_sb[:, lo:], lg[:, lo:], AF.Exp)
                        # running row-sum of p on DVE (replaces the PE's
                        # ones-column l matmul; partition-reduced on GPSIMD
                        # after the tile loop)
                        if idx == 0:
                            nc.vector.tensor_copy(psum_t[:, :], p_sb[:, :])
                        else:
                            nc.vector.tensor_add(psum_t[:, lo:],
                                                 psum_t[:, lo:], p_sb[:, lo:])
                        pv_q.append((idx, st, lo, p_sb))
                        if len(pv_q) > 2:
                            emit_pv(*pv_q.pop(0))
                    for args in pv_q:
                        emit_pv(*args)
                    # normalize: enc = e / l. l = cross-partition sum of
                    # psum_t, reduced AND broadcast in one GPSIMD op (the
                    # Pool engine is otherwise idle).
                    l_all = sb.tile([P, TJ], f32, tag="lall", bufs=3,
                                    name="lall")
                    nc.gpsimd.partition_all_reduce(l_all[:, :], psum_t[:, :],
                                                   P, ReduceOp.add)
                    binv = sb.tile([P, TJ], f32, tag="binv", bufs=2,
                                   name="binv")
                    nc.vector.reciprocal_approx_fast(out=binv[:, :],
                                                     in_=l_all[:, :])
                    nc.vector.tensor_mul(enc[2 * hh][:, tsl], e0[:, :],
                                         binv[:, :])
                    nc.vector.tensor_mul(enc[2 * hh + 1][:, tsl], e1[:, :],
                                         binv[:, :])
                # out-proj deferred by one t-chunk (PE is in-order:
                # emitting it here would stall the next chunk's QK behind
                # the normalize chain)
                if tj > 0:
                    outproj(tj - 1)
                if tj < NTJ - 1:
                    # lazy rope of pair 1 for the NEXT chunk (see wave 2b)
                    rope_pair(q[2], q[3], qraw[2], qraw[3], csq,
                              c0=(tj + 1) * TJ, c1=(tj + 2) * TJ)
            outproj(NTJ - 1)

    nc.compile()
    return nc


def _prep_core_inputs(core, x, wq_eff, wk_eff, wv_eff, wo_eff, cos, sin,
                      attn_mask, bias_list):
    b, j = divmod(core, 4)
    nh0 = j * HEADS_PER_CORE * H  # first flattened q/o column of this core

    xT = np.ascontiguousarray(x[b].T).reshape(DCH, P, T)
    xown = np.ascontiguousarray(xT[:, :, j * TJ:(j + 1) * TJ])

    wq4 = wq_eff.reshape(DCH, P, NH // P, P)
    wq = np.stack([
        np.ascontiguousarray(wq4[:, :, nh0 // P + n, :].transpose(1, 0, 2)
                             ).reshape(P, DCH * P)
        for n in range(NHC)
    ])
    wk4 = wk_eff.reshape(DCH, P, 2, P)
    wk = np.stack([
        np.ascontiguousarray(wk4[:, :, jh, :].transpose(1, 0, 2)
                             ).reshape(P, DCH * P)
        for jh in range(2)
    ])
    wv = wv_eff.reshape(DCH, P, H)
    woc = wo_eff[nh0:nh0 + HEADS_PER_CORE * H, :].reshape(NHC, P, DCH, P)
    wo = np.stack([
        np.ascontiguousarray(woc[:, :, dt, :].transpose(1, 0, 2)
                             ).reshape(P, NHC * P)
        for dt in range(DCH)
    ])

    cb, sbn = cos[b], sin[b]
    scale = float(H) ** -0.5
    csq = np.stack([cb * scale, sbn * scale])
    cskown = np.stack([cb[:, j * TJ:(j + 1) * TJ],
                       sbn[:, j * TJ:(j + 1) * TJ]])

    m = np.asarray(attn_mask)[b, 0]  # [T(query), S(key)]
    if bias_list:
        btiles = []
        for (tj, st, mc) in bias_list:
            sub = m[tj * TJ + mc * P:tj * TJ + (mc + 1) * P,
                    st * P:(st + 1) * P].T  # [s, t]
            btiles.append(np.where(sub, np.float32(0.0), np.float32(BIG_NEG)))
        bias = np.stack(btiles)
    else:
        bias = np.zeros((1, P, P), np.float32)

    cast = lambda a: np.ascontiguousarray(a).astype(_BF16)
    return {
        "xT": cast(xT), "xown": cast(xown), "wq": cast(wq), "wk": cast(wk),
        "wv": cast(wv), "wo": cast(wo), "csq": cast(csq),
        "cskown": cast(cskown), "bias": cast(bias),
    }


def kernel(x, positions, attn_mask, decode, q_w, q_lora_a, q_lora_b,
           kv_w, kv_lora_a, kv_lora_b, out_w, out_lora_a, out_lora_b):
    global LAST_EXEC_TIME_NS
    from concourse.bass_utils import run_bass_kernel_spmd

    x = np.asarray(x, np.float32)
    positions = np.asarray(positions)
    attn_mask = np.asarray(attn_mask)

    wq_eff, wk_eff, wv_eff, wo_eff = _fold_weights(
        np.asarray(q_w, np.float32), np.asarray(q_lora_a, np.float32),
        np.asarray(q_lora_b, np.float32), np.asarray(kv_w, np.float32),
        np.asarray(kv_lora_a, np.float32), np.asarray(kv_lora_b, np.float32),
        np.asarray(out_w, np.float32), np.asarray(out_lora_a, np.float32),
        np.asarray(out_lora_b, np.float32))

    cos, sin = [], []
    for b in range(B):
        c_, s_ = _rope_tables(positions[b])
        cos.append(c_)
        sin.append(s_)

    s_tiles, lo_cols, bias_list = _classify_mask(attn_mask)
    nc = _build_program(s_tiles, lo_cols, bias_list)

    in_maps = [
        _prep_core_inputs(core, x, wq_eff, wk_eff, wv_eff, wo_eff, cos, sin,
                          attn_mask, bias_list)
        for core in range(N_CORES)
    ]

    trace = os.environ.get("KERNEL_PROFILE", "0") == "1"
    if trace:
        try:
            import antenv.axon_hooks  # noqa: F401
        except ImportError:
            trace = False
    res = run_bass_kernel_spmd(nc, in_maps, core_ids=list(range(N_CORES)),
                               trace=trace)
    LAST_EXEC_TIME_NS = res.exec_time_ns
    globals()["LAST_RESULT"] = res

    out = np.zeros((B, T, D), np.float32)
    for core in range(N_CORES):
        b = core // 4
        part = res.results[core]["out"].reshape(D, T)  # [d, t] bf16
        out[b] += part.T.astype(np.float32)
    return out

